# revision 5
# baseline (speedup 1.0000x reference)
"""Supervised contrastive loss on 8 Trainium2 NeuronCores.

Strategy (data-parallel over embedding rows, per the sharding hint):
  - Each core owns 512 rows of the [4096, 512] embedding matrix and computes
    its [512, 4096] similarity slab against the full embedding matrix with
    PE matmuls (lhsT = E_local^T, rhs = E_full^T, both shipped pre-transposed
    from the host so no on-device transpose is needed).
  - exp(sims/T) is computed on ACT directly out of PSUM with a fused
    per-row accumulate (row sum of exponentials).
  - The same-label mask is built on GPSIMD with a tensor_scalar is_equal
    against a broadcast column-label tile.
  - A single DVE tensor_tensor_reduce produces both the masked exponentials
    (me = exp * mask) and their row sum in one pass.
  - Using  sum_{j in same} log(denom_i + E_ij)
             = cnt_i * ln(denom_i) + sum_j log1p(me_ij / denom_i)
    the masked log-sum becomes ONE dense ACT Ln pass (scale = 1/denom per
    partition, bias = 1) with a free fused row-accumulate; the mask is
    absorbed because log1p(0) = 0.
  - sum_{j in same} sims_ij collapses through the label one-hot:
    sum_j same_ij s_ij = E_i . G_{label_i} / T with G = class-sums of E,
    an O(B*D) host precompute, shipped per-row. Diagonal terms are removed
    per-row with host-shipped ||E_i||^2.
  - Each core writes its 512 per-row loss contributions; the host sums the
    4096 values and divides by num_pos (exact, from label counts).
"""

import numpy as np

import concourse.bass as bass
import concourse.bacc as bacc
import concourse.mybir as mybir
import concourse.tile as tile
from concourse.bass_utils import run_bass_kernel_spmd

B = 4096          # total rows
D = 512           # embedding dim
NCORES = 8
BL = B // NCORES  # rows per core
NK = D // 128     # contraction k-tiles
NMT = BL // 128   # output m-tiles per core
CH = 2048         # column chunk (4 PSUM banks)
NCH = B // CH     # chunks per m-tile row
TINV = 10.0       # 1 / temperature
F32 = mybir.dt.float32

_CACHE = {}


def _build_nc():
    nc = bacc.Bacc()
    et = nc.dram_tensor("et", [D, B], F32, kind="ExternalInput")
    elt = nc.dram_tensor("elt", [D, BL], F32, kind="ExternalInput")
    collab = nc.dram_tensor("collab", [128, B], F32, kind="ExternalInput")
    meta = nc.dram_tensor("meta", [NMT, 128, 4], F32, kind="ExternalInput")
    out = nc.dram_tensor("out", [NMT, 128, 1], F32, kind="ExternalOutput")

    AF = mybir.ActivationFunctionType
    OP = mybir.AluOpType

    with tile.TileContext(nc) as tc:
        with (
            tc.tile_pool(name="const", bufs=1) as cpool,
            tc.tile_pool(name="psum", bufs=2, space=bass.MemorySpace.PSUM) as ppool,
            tc.tile_pool(name="chunks", bufs=3) as chpool,
            tc.tile_pool(name="me", bufs=2) as mepool,
            tc.tile_pool(name="scratch", bufs=1) as spool,
            tc.tile_pool(name="small", bufs=2) as smpool,
        ):
            ets = [cpool.tile([128, B], F32, tag=f"ets{k}", name=f"ets{k}") for k in range(NK)]
            eltt = [cpool.tile([128, BL], F32, tag=f"elt{k}", name=f"elt{k}") for k in range(NK)]
            collab_sb = cpool.tile([128, B], F32, tag="collab")
            meta_sb = [cpool.tile([128, 4], F32, tag=f"meta{m}", name=f"meta{m}") for m in range(NMT)]
            lnout = spool.tile([128, B], F32, tag="lnout")

            # Loads, chunk-major so the first-needed columns land first.
            for k in range(NK):
                nc.sync.dma_start(eltt[k][:], elt[k * 128:(k + 1) * 128, :])
            for c in range(NCH):
                for k in range(NK):
                    nc.sync.dma_start(
                        ets[k][:, c * CH:(c + 1) * CH],
                        et[k * 128:(k + 1) * 128, c * CH:(c + 1) * CH],
                    )
            nc.sync.dma_start(collab_sb[:], collab[:])
            for m in range(NMT):
                nc.sync.dma_start(meta_sb[m][:], meta[m])

            for mt in range(NMT):
                rowlab = meta_sb[mt][:, 0:1]
                cnt = meta_sb[mt][:, 1:2]
                sii = meta_sb[mt][:, 2:3]
                rds = meta_sb[mt][:, 3:4]

                me = mepool.tile([128, B], F32, tag="me")
                separts = smpool.tile([128, NCH], F32, tag="separts")
                ssparts = smpool.tile([128, NCH], F32, tag="ssparts")

                for c in range(NCH):
                    psum = ppool.tile([128, CH], F32, tag="psum")
                    for k in range(NK):
                        lhsT = eltt[k][:, mt * 128:(mt + 1) * 128]
                        for h in range(CH // 512):
                            col0 = c * CH + h * 512
                            nc.tensor.matmul(
                                psum[:, h * 512:(h + 1) * 512],
                                lhsT,
                                ets[k][:, col0:col0 + 512],
                                start=(k == 0),
                                stop=(k == NK - 1),
                            )
                    expt = chpool.tile([128, CH], F32, tag="exp")
                    nc.scalar.activation(
                        expt[:], psum[:], AF.Exp,
                        scale=TINV, accum_out=separts[:, c:c + 1],
                    )
                    maskt = chpool.tile([128, CH], F32, tag="mask")
                    nc.vector.tensor_scalar(
                        maskt[:], collab_sb[:, c * CH:(c + 1) * CH],
                        rowlab, None, OP.is_equal,
                    )
                    nc.gpsimd.tensor_tensor(
                        me[:, c * CH:(c + 1) * CH], expt[:], maskt[:], OP.mult)
                    nc.vector.tensor_reduce(
                        ssparts[:, c:c + 1], me[:, c * CH:(c + 1) * CH],
                        mybir.AxisListType.X, OP.add)

                sum_exp = smpool.tile([128, 1], F32, tag="sum_exp")
                nc.vector.tensor_reduce(
                    sum_exp[:], separts[:], mybir.AxisListType.X, OP.add)
                sum_same = smpool.tile([128, 1], F32, tag="sum_same")
                nc.vector.tensor_reduce(
                    sum_same[:], ssparts[:], mybir.AxisListType.X, OP.add)
                denom = smpool.tile([128, 1], F32, tag="denom")
                nc.vector.tensor_sub(denom[:], sum_exp[:], sum_same[:])
                inv = smpool.tile([128, 1], F32, tag="inv")
                nc.vector.reciprocal(inv[:], denom[:])
                lnden = smpool.tile([128, 1], F32, tag="lnden")
                nc.scalar.activation(lnden[:], denom[:], AF.Ln)
                eii = smpool.tile([128, 1], F32, tag="eii")
                nc.scalar.activation(eii[:], sii, AF.Exp)
                lndiag = smpool.tile([128, 1], F32, tag="lndiag")
                nc.scalar.activation(lndiag[:], eii[:], AF.Ln, bias=denom[:])

                # rowconst = cnt*lnden - rds - lndiag + sii
                t1 = smpool.tile([128, 1], F32, tag="t1")
                nc.vector.tensor_tensor(t1[:], cnt, lnden[:], OP.mult)
                t2 = smpool.tile([128, 1], F32, tag="t2")
                nc.vector.tensor_sub(t2[:], t1[:], rds)
                t3 = smpool.tile([128, 1], F32, tag="t3")
                nc.vector.tensor_sub(t3[:], t2[:], lndiag[:])
                rc = smpool.tile([128, 1], F32, tag="rc")
                nc.vector.tensor_add(rc[:], t3[:], sii)

                slog = smpool.tile([128, 1], F32, tag="slog")
                nc.scalar.activation(
                    lnout[:], me[:], AF.Ln,
                    scale=inv[:], bias=1.0, accum_out=slog[:],
                )
                rowtot = smpool.tile([128, 1], F32, tag="rowtot")
                nc.vector.tensor_add(rowtot[:], rc[:], slog[:])
                nc.sync.dma_start(out[mt], rowtot[:])
    nc.compile()
    return nc


def kernel(embeddings, labels):
    emb = np.ascontiguousarray(np.asarray(embeddings, dtype=np.float32))
    lab = np.asarray(labels).astype(np.int64)
    assert emb.shape == (B, D) and lab.shape == (B,)

    ET = np.ascontiguousarray(emb.T)                      # [D, B]
    labf = lab.astype(np.float32)
    collab = np.ascontiguousarray(np.broadcast_to(labf[None, :], (128, B)))

    counts = np.bincount(lab, minlength=int(lab.max()) + 1)
    cnt = counts[lab].astype(np.float64)                  # same-label count incl. self
    num_pos = float(cnt.sum() - B)

    emb64 = emb.astype(np.float64)
    G = np.zeros((counts.size, D), np.float64)
    np.add.at(G, lab, emb64)
    rds = (emb64 * G[lab]).sum(1) * TINV                  # sum_{j same} sims_ij / T
    sii = (emb64 * emb64).sum(1) * TINV                   # sims_ii / T

    meta_all = np.stack(
        [labf.astype(np.float64), cnt, sii, rds], axis=-1
    ).astype(np.float32)                                  # [B, 4]

    if "nc" not in _CACHE:
        _CACHE["nc"] = _build_nc()
    nc = _CACHE["nc"]

    in_maps = []
    for c in range(NCORES):
        sl = slice(c * BL, (c + 1) * BL)
        in_maps.append({
            "et": ET,
            "elt": np.ascontiguousarray(ET[:, sl]),
            "collab": collab,
            "meta": np.ascontiguousarray(meta_all[sl].reshape(NMT, 128, 4)),
        })

    res = run_bass_kernel_spmd(nc, in_maps, list(range(NCORES)))
    total = sum(float(r["out"].sum()) for r in res.results)
    return np.asarray(total / max(num_pos, 1.0), dtype=np.float32)


# revision 11
# speedup vs baseline: 1.5839x; 1.5839x over previous
"""Supervised contrastive loss on 8 Trainium2 NeuronCores.

Strategy (data-parallel over embedding rows, per the sharding hint):
  - Each core owns 512 rows of the [4096, 512] embedding matrix and computes
    its [512, 4096] similarity slab against the full embedding matrix with
    PE matmuls (lhsT = E_local^T, rhs = E_full^T, both shipped pre-transposed
    from the host so no on-device transpose is needed).
  - exp(sims/T) is computed on ACT directly out of PSUM with a fused
    per-row accumulate (row sum of exponentials).
  - The same-label mask is built on GPSIMD with a tensor_scalar is_equal
    against a broadcast column-label tile.
  - A single DVE tensor_tensor_reduce produces both the masked exponentials
    (me = exp * mask) and their row sum in one pass.
  - Using  sum_{j in same} log(denom_i + E_ij)
             = cnt_i * ln(denom_i) + sum_j log1p(me_ij / denom_i)
    the masked log-sum becomes ONE dense ACT Ln pass (scale = 1/denom per
    partition, bias = 1) with a free fused row-accumulate; the mask is
    absorbed because log1p(0) = 0.
  - sum_{j in same} sims_ij collapses through the label one-hot:
    sum_j same_ij s_ij = E_i . G_{label_i} / T with G = class-sums of E,
    an O(B*D) host precompute, shipped per-row. Diagonal terms are removed
    per-row with host-shipped ||E_i||^2.
  - Each core writes its 512 per-row loss contributions; the host sums the
    4096 values and divides by num_pos (exact, from label counts).
"""

import ml_dtypes
import numpy as np

import concourse.bass as bass
import concourse.bacc as bacc
import concourse.mybir as mybir
import concourse.tile as tile
from concourse.bass_utils import run_bass_kernel_spmd

B = 4096          # total rows
D = 512           # embedding dim
NCORES = 8
BL = B // NCORES  # rows per core
NK = D // 128     # contraction k-tiles
NMT = BL // 128   # output m-tiles per core
CH = 2048         # column chunk (4 PSUM banks)
NCH = B // CH     # chunks per m-tile row
TINV = 10.0       # 1 / temperature
F32 = mybir.dt.float32
BF16 = mybir.dt.bfloat16

_CACHE = {}


def _build_nc():
    nc = bacc.Bacc()
    et = nc.dram_tensor("et", [D, B], BF16, kind="ExternalInput")
    elt = nc.dram_tensor("elt", [D, BL], BF16, kind="ExternalInput")
    collab = nc.dram_tensor("collab", [128, B], BF16, kind="ExternalInput")
    meta = nc.dram_tensor("meta", [NMT, 128, 5], F32, kind="ExternalInput")
    out = nc.dram_tensor("out", [NMT, 128, 1], F32, kind="ExternalOutput")

    AF = mybir.ActivationFunctionType
    OP = mybir.AluOpType

    with tile.TileContext(nc) as tc:
        with (
            tc.tile_pool(name="const", bufs=1) as cpool,
            tc.tile_pool(name="psum", bufs=2, space=bass.MemorySpace.PSUM) as ppool,
            tc.tile_pool(name="chunks", bufs=3) as chpool,
            tc.tile_pool(name="me", bufs=1) as mepool,
            tc.tile_pool(name="scratch", bufs=1) as spool,
            tc.tile_pool(name="small", bufs=2) as smpool,
        ):
            ets = [cpool.tile([128, B], BF16, tag=f"ets{k}", name=f"ets{k}") for k in range(NK)]
            eltt = [cpool.tile([128, BL], BF16, tag=f"elt{k}", name=f"elt{k}") for k in range(NK)]
            collab_sb = cpool.tile([128, B], BF16, tag="collab")
            meta_sb = [cpool.tile([128, 5], F32, tag=f"meta{m}", name=f"meta{m}") for m in range(NMT)]
            lnout = spool.tile([128, B], F32, tag="lnout")

            # Loads, chunk-major so the first-needed columns land first.
            for k in range(NK):
                nc.sync.dma_start(eltt[k][:], elt[k * 128:(k + 1) * 128, :])
            for c in range(NCH):
                for k in range(NK):
                    nc.sync.dma_start(
                        ets[k][:, c * CH:(c + 1) * CH],
                        et[k * 128:(k + 1) * 128, c * CH:(c + 1) * CH],
                    )
            nc.sync.dma_start(collab_sb[:], collab[:])
            for m in range(NMT):
                nc.sync.dma_start(meta_sb[m][:], meta[m])

            mes, invs, denoms = [], [], []
            for mt in range(NMT):
                rowlab = meta_sb[mt][:, 0:1]

                me = mepool.tile([128, B], F32, tag=f"me{mt}", name=f"me{mt}")
                separts = smpool.tile([128, NCH], F32, tag="separts")
                ssparts = smpool.tile([128, NCH], F32, tag="ssparts")

                for c in range(NCH):
                    psum = ppool.tile([128, CH], F32, tag="psum")
                    for k in range(NK):
                        lhsT = eltt[k][:, mt * 128:(mt + 1) * 128]
                        for h in range(CH // 512):
                            col0 = c * CH + h * 512
                            nc.tensor.matmul(
                                psum[:, h * 512:(h + 1) * 512],
                                lhsT,
                                ets[k][:, col0:col0 + 512],
                                start=(k == 0),
                                stop=(k == NK - 1),
                            )
                    expt = chpool.tile([128, CH], F32, tag="exp")
                    nc.scalar.activation(
                        expt[:], psum[:], AF.Exp,
                        scale=TINV, accum_out=separts[:, c:c + 1],
                    )
                    maskt = chpool.tile([128, CH], BF16, tag="mask")
                    nc.vector.tensor_scalar(
                        maskt[:], collab_sb[:, c * CH:(c + 1) * CH],
                        rowlab, None, OP.is_equal,
                    )
                    # split the masked multiply between Pool and DVE
                    tt_eng = nc.vector if (mt * NCH + c) % 4 == 3 else nc.gpsimd
                    tt_eng.tensor_tensor(
                        me[:, c * CH:(c + 1) * CH], expt[:], maskt[:], OP.mult)
                    nc.vector.tensor_reduce(
                        ssparts[:, c:c + 1], me[:, c * CH:(c + 1) * CH],
                        mybir.AxisListType.X, OP.add)

                sum_exp = smpool.tile([128, 1], F32, tag="sum_exp")
                nc.vector.tensor_reduce(
                    sum_exp[:], separts[:], mybir.AxisListType.X, OP.add)
                sum_same = smpool.tile([128, 1], F32, tag="sum_same")
                nc.vector.tensor_reduce(
                    sum_same[:], ssparts[:], mybir.AxisListType.X, OP.add)
                denom = smpool.tile([128, 1], F32, tag=f"denom{mt}", name=f"denom{mt}")
                nc.vector.tensor_sub(denom[:], sum_exp[:], sum_same[:])
                inv = smpool.tile([128, 1], F32, tag=f"inv{mt}", name=f"inv{mt}")
                nc.vector.reciprocal(inv[:], denom[:])
                mes.append(me); invs.append(inv); denoms.append(denom)

            # Phase B: all Ln-family work batched so the ACT table set
            # switches once (Exp set -> Ln set) instead of per m-tile.
            for mt in range(NMT):
                cnt = meta_sb[mt][:, 1:2]
                sii = meta_sb[mt][:, 2:3]
                rds = meta_sb[mt][:, 3:4]
                eii = meta_sb[mt][:, 4:5]
                me, inv, denom = mes[mt], invs[mt], denoms[mt]

                lnden = smpool.tile([128, 1], F32, tag="lnden")
                nc.scalar.activation(lnden[:], denom[:], AF.Ln)
                lndiag = smpool.tile([128, 1], F32, tag="lndiag")
                nc.scalar.activation(lndiag[:], eii, AF.Ln, bias=denom[:])

                # rowconst = cnt*lnden - rds - lndiag + sii
                t1 = smpool.tile([128, 1], F32, tag="t1")
                nc.vector.tensor_tensor(t1[:], cnt, lnden[:], OP.mult)
                t2 = smpool.tile([128, 1], F32, tag="t2")
                nc.vector.tensor_sub(t2[:], t1[:], rds)
                t3 = smpool.tile([128, 1], F32, tag="t3")
                nc.vector.tensor_sub(t3[:], t2[:], lndiag[:])
                rc = smpool.tile([128, 1], F32, tag="rc")
                nc.vector.tensor_add(rc[:], t3[:], sii)

                slog = smpool.tile([128, 1], F32, tag="slog")
                nc.scalar.activation(
                    lnout[:], me[:], AF.Ln,
                    scale=inv[:], bias=1.0, accum_out=slog[:],
                )
                rowtot = smpool.tile([128, 1], F32, tag="rowtot")
                nc.vector.tensor_add(rowtot[:], rc[:], slog[:])
                nc.sync.dma_start(out[mt], rowtot[:])
    nc.compile()
    return nc


def _make_in_maps(embeddings, labels):
    """Host-side prep: transposes, per-row scalars, per-core input dicts.
    Returns (in_maps, num_pos)."""
    emb = np.ascontiguousarray(np.asarray(embeddings, dtype=np.float32))
    lab = np.asarray(labels).astype(np.int64)
    assert emb.shape == (B, D) and lab.shape == (B,)

    ET = np.ascontiguousarray(emb.T)                      # [D, B]
    labf = lab.astype(np.float32)
    lab16 = labf.astype(ml_dtypes.bfloat16)
    collab = np.ascontiguousarray(np.broadcast_to(lab16[None, :], (128, B)))

    counts = np.bincount(lab, minlength=int(lab.max()) + 1)
    cnt = counts[lab].astype(np.float64)                  # same-label count incl. self
    num_pos = float(cnt.sum() - B)

    emb64 = emb.astype(np.float64)
    G = np.zeros((counts.size, D), np.float64)
    np.add.at(G, lab, emb64)
    rds = (emb64 * G[lab]).sum(1) * TINV                  # sum_{j same} sims_ij / T
    sii = (emb64 * emb64).sum(1) * TINV                   # sims_ii / T

    meta_all = np.stack(
        [labf.astype(np.float64), cnt, sii, rds, np.exp(sii)], axis=-1
    ).astype(np.float32)                                  # [B, 5]

    ET16 = ET.astype(ml_dtypes.bfloat16)
    in_maps = []
    for c in range(NCORES):
        sl = slice(c * BL, (c + 1) * BL)
        in_maps.append({
            "et": ET16,
            "elt": np.ascontiguousarray(ET16[:, sl]),
            "collab": collab,
            "meta": np.ascontiguousarray(meta_all[sl].reshape(NMT, 128, 5)),
        })
    return in_maps, num_pos


def kernel(embeddings, labels):
    in_maps, num_pos = _make_in_maps(embeddings, labels)
    if "nc" not in _CACHE:
        _CACHE["nc"] = _build_nc()
    nc = _CACHE["nc"]
    res = run_bass_kernel_spmd(nc, in_maps, list(range(NCORES)))
    total = sum(float(r["out"].sum()) for r in res.results)
    return np.asarray(total / max(num_pos, 1.0), dtype=np.float32)
